# revision 16
# baseline (speedup 1.0000x reference)
"""Multi-head attention (B=2, S=2048, D=768, H=12) on 8 TRN2 NeuronCores.

Sharding: data-parallel over batch (2) x tensor-parallel over heads
(4 groups of 3 heads), Megatron-style. Core c handles batch c//4 and
heads 3*(c%4) .. 3*(c%4)+2. Each core computes a partial (S, D) output
(its heads' contribution through Wo); the host sums the 4 partials per
batch. bo is added on exactly one core per batch (the others get zeros).

Device kernel (per core), all matmuls bf16 with fp32 PSUM accumulation.
PE cost on TRN2 is purely N-columns at 0.4167ns/col when streaming, so
the design minimizes total moving columns and keeps the PE queue fed:

  wqkv columns: [k01 | kq2 | q01 | v] where kq2 = [Wk[:,128:192] |
  Wq[:,128:192]] packs head-2's k and q into ONE projection M-tile
  (3 M-tiles for Q+K instead of 4). q2 rows (partitions 64:128 of the
  kq2 projection) are moved to partition base 0 by an SBUF->SBUF DMA so
  score matmuls for head 2 have matching lhsT/rhs partition bases.

  DMA order is pipeline-critical: k01 cols + biases, xT qb0, q01 cols,
  v cols, xT qb1, wo/bo. First score matmul can issue ~6us in.

  attention pass per (qb, head): per k-tile: scoresT = k @ qT (PSUM,
  [128,1024]) -> exp on ACT (bf16 'at' tile) -> [v|1]^T @ attnT
  accumulated in two [65,512] PSUM accumulators (row 64 = softmax
  denominator via the ones column of vv). Denominator: DVE copy ->
  SBUF->SBUF DMA to partition 0 -> reciprocal -> gpsimd broadcast ->
  fused normalize multiply into outT (bf16).

  out-projection per 128-row block: qb0 blocks are unified (3 head
  matmuls in one PSUM group + single DVE add of bo) and demoted so the
  scheduler uses them as PE filler inside qb1's ACT-bound windows; qb1
  keeps the ab/c split (h0+h1 early as filler during h2's attention,
  h2 contribution + final add in the tail).
"""

import numpy as np
import ml_dtypes

BF16 = ml_dtypes.bfloat16

B, S, D = 2, 2048, 768
H, HD = 12, 64
HPC = 3            # heads per core
DC = HPC * HD      # 192 projection columns per core
NKT = S // 128     # 16 k-tiles
NDT = D // 128     # 6 contraction tiles for projections
QB = 1024          # q-block width for scores/exp
NQB = S // QB      # 2
WCOL = 3 * 128 + DC  # wqkv columns: k01 | kq2 | q01 | v

_cache = {}


def _build_nc():
    import concourse.bacc as bacc
    import concourse.mybir as mybir
    import concourse.tile as tile

    f32 = mybir.dt.float32
    bf16 = mybir.dt.bfloat16
    Exp = mybir.ActivationFunctionType.Exp

    nc = bacc.Bacc("TRN2", target_bir_lowering=False, debug=False, num_devices=1)

    xT = nc.dram_tensor("xT", (D, S), bf16, kind="ExternalInput")
    wqkv = nc.dram_tensor("wqkv", (D, WCOL), bf16, kind="ExternalInput")
    wo = nc.dram_tensor("wo", (HD, HPC, D), bf16, kind="ExternalInput")
    bqk0 = nc.dram_tensor("bqk0", (128, 2), f32, kind="ExternalInput")
    bkq2 = nc.dram_tensor("bkq2", (128, 1), f32, kind="ExternalInput")
    bv = nc.dram_tensor("bv", (1, DC), bf16, kind="ExternalInput")
    bo_t = nc.dram_tensor("bo_t", (128, D), f32, kind="ExternalInput")
    out = nc.dram_tensor("out", (S, D), bf16, kind="ExternalOutput")

    with tile.TileContext(nc) as tc:
        with (
            tc.tile_pool(name="persist", bufs=1) as sbp,
            tc.tile_pool(name="att", bufs=6) as att,
            tc.tile_pool(name="stagp", bufs=2) as stagp,
            tc.tile_pool(name="dbcp", bufs=3) as dbcp,
            tc.tile_pool(name="orwp", bufs=4) as orwp,
            tc.tile_pool(name="osbp", bufs=10) as osbp,
            tc.tile_pool(name="scp", bufs=2, space="PSUM") as scp,
            tc.tile_pool(name="acp", bufs=1, space="PSUM") as acp,
            tc.tile_pool(name="pjp", bufs=2, space="PSUM") as pjp,
        ):
            # ---- persistent SBUF tensors ----
            xT_sb = []
            wqkv_sb = []
            for kt in range(NDT):
                xT_sb.append(sbp.tile([128, S], bf16, name=f"xT_sb{kt}"))
                wqkv_sb.append(sbp.tile([128, WCOL], bf16, name=f"wqkv_sb{kt}"))
            bqk0_sb = sbp.tile([128, 2], f32)
            bkq2_sb = sbp.tile([128, 1], f32)
            bv_sb = sbp.tile([1, DC], bf16)
            wo_sb = sbp.tile([HD, HPC, D], bf16)
            bo_sb = sbp.tile([128, D], f32)

            # ---- input DMAs: multi-engine dispatch (each engine's queue
            # dispatches serially at ~620ns/DMA; DMA can only issue from
            # sync/scalar/gpsimd, so the head transfers are balanced
            # across those three; wqkv is laid out [k01|q01|v|kq2] so
            # q01+v+kq2 is one contiguous DMA per contraction tile) ----
            def dslice(kt):
                return slice(kt * 128, (kt + 1) * 128)

            # wave 1: xT qb0 first halves (sync), k01 cols (gpsimd),
            # biases then fused q01/v/kq2 cols (scalar)
            for kt in range(NDT):
                nc.sync.dma_start(out=xT_sb[kt][:, 0:512],
                                  in_=xT.ap()[dslice(kt), 0:512])
            for kt in range(NDT):
                nc.gpsimd.dma_start(out=wqkv_sb[kt][:, 0:128],
                                    in_=wqkv.ap()[dslice(kt), 0:128])
            nc.scalar.dma_start(out=bqk0_sb, in_=bqk0.ap())
            nc.scalar.dma_start(out=bkq2_sb, in_=bkq2.ap())
            nc.scalar.dma_start(out=bv_sb, in_=bv.ap())
            for kt in range(NDT):
                nc.scalar.dma_start(out=wqkv_sb[kt][:, 128:WCOL],
                                    in_=wqkv.ap()[dslice(kt), 128:WCOL])
            # wave 2: xT qb0 second halves (gpsimd), xT qb1 (sync+scalar)
            for kt in range(NDT):
                nc.gpsimd.dma_start(out=xT_sb[kt][:, 512:QB],
                                    in_=xT.ap()[dslice(kt), 512:QB])
            for kt in range(3):
                nc.sync.dma_start(out=xT_sb[kt][:, QB:S],
                                  in_=xT.ap()[dslice(kt), QB:S])
            for kt in range(3, NDT):
                nc.scalar.dma_start(out=xT_sb[kt][:, QB:S],
                                    in_=xT.ap()[dslice(kt), QB:S])
            # wave 3: wo, bo (needed tens of us in)
            nc.sync.dma_start(out=wo_sb, in_=wo.ap())
            nc.sync.dma_start(out=bo_sb, in_=bo_t.ap())

            ones_row = sbp.tile([1, 128], bf16)
            nc.vector.memset(ones_row, 1.0)

            # warm up the ACT exp table early (overlaps the input DMAs)
            wu = sbp.tile([1, 8], f32)
            nc.vector.memset(wu, 0.0)
            wu2 = sbp.tile([1, 8], f32)
            nc.scalar.activation(wu2, wu, Exp, scale=1.0)

            qT0 = sbp.tile([128, S], bf16)   # q heads 0 (p0:64) / 1 (p64:128)
            kT0 = sbp.tile([128, S], bf16)   # k heads 0 / 1
            kq2T = sbp.tile([128, S], bf16)  # k2 (p0:64) / q2 (p64:128)
            q2Ts = sbp.tile([64, S], bf16)   # q2 shifted to partition base 0
            # v natural: [v | ones] -> M=65 (denominator row 64)
            vv = [sbp.tile([128, HPC, 65], bf16, name=f"vv{st}")
                  for st in range(NKT)]
            outT = sbp.tile([HD, HPC, S], bf16)  # normalized outT
            drow = sbp.tile([1, HPC * S], f32)
            drec = sbp.tile([1, HPC * S], f32)

            # ---- phase 1: projections ----
            def qkproj(qb, name):
                col0, dest, bias_sb, bcol, shift = {
                    "k01": (0, kT0, bqk0_sb, 1, False),
                    "q01": (128, qT0, bqk0_sb, 0, False),
                    "kq2": (448, kq2T, bkq2_sb, 0, True),
                }[name]
                for half in range(2):
                    qs = slice(qb * QB + half * 512,
                               qb * QB + (half + 1) * 512)
                    ps = pjp.tile([128, 512], f32, name="pj", tag="pj")
                    for kt in range(NDT):
                        nc.tensor.matmul(
                            ps, wqkv_sb[kt][:, col0:col0 + 128],
                            xT_sb[kt][:, qs],
                            start=(kt == 0), stop=(kt == NDT - 1))
                    nc.vector.tensor_scalar_add(
                        dest[:, qs], ps, bias_sb[:, bcol:bcol + 1])
                    if shift:
                        # move q2 rows to partition base 0 so head-2 score
                        # matmuls have lhsT/rhs at the same base partition
                        nc.sync.dma_start(out=q2Ts[:, qs],
                                          in_=dest[64:128, qs])

            def vproj(st):
                ss = slice(st * 128, (st + 1) * 128)
                vps = pjp.tile([128, DC], f32, name="vps", tag="pj")
                for kt in range(NDT):
                    nc.tensor.matmul(
                        vps, xT_sb[kt][:, ss], wqkv_sb[kt][:, 256:448],
                        start=(kt == 0), stop=False)
                nc.tensor.matmul(vps, ones_row, bv_sb, start=False, stop=True)
                nc.vector.tensor_copy(
                    vv[st][:, :, 0:HD],
                    vps.rearrange("p (h d) -> p h d", h=HPC))
                nc.vector.memset(vv[st][:, :, HD:HD + 1], 1.0)

            # critical projections at natural priority, the rest demoted so
            # the attention stream preempts as soon as inputs are ready and
            # projections fill PE idle slots inside ACT-bound windows.
            demote = dict(offset=-1_000_000)
            qkproj(0, "k01")
            qkproj(0, "q01")
            for st in range(8):
                vproj(st)
            with tc.high_priority(**demote):
                qkproj(1, "k01")          # needed by exp slot 8 of qb0 pass
            for st in range(8, NKT):
                with tc.high_priority(**demote):
                    vproj(st)
            # emission order must match need-order (pool ring slots are
            # assigned in emission order): q01-qb1 gates pass 3 = (1,0);
            # kq2 (both halves) gates pass 4 = (0,2).
            with tc.high_priority(**demote):
                qkproj(1, "q01")
                qkproj(0, "kq2")
                qkproj(1, "kq2")

            # ---- phase 2: attention ----
            def attn_head_pass(qb, h):
                qs = slice(qb * QB, (qb + 1) * QB)
                if h == 0:
                    kh, qh = kT0[0:HD, :], qT0[0:HD, :]
                elif h == 1:
                    kh, qh = kT0[HD:128, :], qT0[HD:128, :]
                else:
                    kh, qh = kq2T[0:HD, :], q2Ts
                acc = acp.tile([65, QB], f32, name="acc", tag="ac")
                sc_t = {}

                def emit_scores(kt):
                    sc = scp.tile([128, QB], f32, name="sc", tag="sc")
                    for half in range(2):
                        hs = slice(qb * QB + half * 512,
                                   qb * QB + (half + 1) * 512)
                        nc.tensor.matmul(sc[:, half * 512:(half + 1) * 512],
                                         kh[:, kt * 128:(kt + 1) * 128],
                                         qh[:, hs], start=True, stop=True)
                    sc_t[kt] = sc

                with tc.high_priority():
                    emit_scores(0)
                for kt in range(NKT):
                    at = att.tile([128, QB], bf16, name="at", tag="at")
                    nc.scalar.activation(at, sc_t.pop(kt), Exp, scale=0.125)
                    if kt + 1 < NKT:
                        emit_scores(kt + 1)
                    for i in range(2):
                        nc.tensor.matmul(
                            acc[:, i * 512:(i + 1) * 512], vv[kt][:, h, :],
                            at[:, i * 512:(i + 1) * 512],
                            start=(kt == 0), stop=(kt == NKT - 1))
                # release accumulators with raw copies, normalize after
                orw = orwp.tile([HD, QB], bf16, name="orw", tag="orw")
                stg = stagp.tile([128, QB], f32, name="stg", tag="stg")
                off = h * S + qb * QB
                nc.vector.tensor_copy(orw, acc[0:HD, :])
                nc.vector.tensor_copy(stg[64:65, :], acc[64:65, :])
                nc.sync.dma_start(out=drow[0:1, off:off + QB],
                                  in_=stg[64:65, :])
                nc.vector.reciprocal_approx_fast(
                    drec[0:1, off:off + QB], drow[0:1, off:off + QB])
                dbc = dbcp.tile([HD, QB], f32, name="dbc", tag="dbc")
                nc.gpsimd.partition_broadcast(
                    dbc, drec[0:1, off:off + QB], channels=HD)
                nc.vector.tensor_mul(outT[:, h, qs], orw, dbc)

            # ---- phase 3: output projection ----
            # (gpsimd cannot read PSUM on TRN2, so the P+bias adds must
            # all go through DVE)
            def veng(sub, c):
                return nc.vector

            def out_block_unified(sub):
                rs = slice(sub * 128, (sub + 1) * 128)
                osb = osbp.tile([128, D], bf16, name="osb", tag="osb")
                for c in range(2):
                    cs2 = slice(c * 384, (c + 1) * 384)
                    P = pjp.tile([128, 384], f32, name="P", tag="pj")
                    for h in range(HPC):
                        nc.tensor.matmul(P, outT[:, h, rs], wo_sb[:, h, cs2],
                                         start=(h == 0), stop=(h == HPC - 1))
                    veng(sub, c).tensor_add(osb[:, cs2], P, bo_sb[:, cs2])
                nc.sync.dma_start(out=out.ap()[rs, :], in_=osb)

            acc_sbs = {}

            def out_block_ab(sub):
                rs = slice(sub * 128, (sub + 1) * 128)
                acc_sb = osbp.tile([128, D], bf16, name="acc_sb", tag="osb")
                acc_sbs[sub] = acc_sb
                for c in range(2):
                    cs2 = slice(c * 384, (c + 1) * 384)
                    P = pjp.tile([128, 384], f32, name="P", tag="pj")
                    nc.tensor.matmul(P, outT[:, 0, rs], wo_sb[:, 0, cs2],
                                     start=True, stop=False)
                    nc.tensor.matmul(P, outT[:, 1, rs], wo_sb[:, 1, cs2],
                                     start=False, stop=True)
                    veng(sub, c).tensor_add(acc_sb[:, cs2], P, bo_sb[:, cs2])

            def out_block_c(sub):
                rs = slice(sub * 128, (sub + 1) * 128)
                acc_sb = acc_sbs.pop(sub)
                for c in range(2):
                    cs2 = slice(c * 384, (c + 1) * 384)
                    P = pjp.tile([128, 384], f32, name="P2", tag="pj")
                    nc.tensor.matmul(P, outT[:, 2, rs], wo_sb[:, 2, cs2],
                                     start=True, stop=True)
                    veng(sub, c).tensor_add(acc_sb[:, cs2], P, acc_sb[:, cs2])
                nc.sync.dma_start(out=out.ap()[rs, :], in_=acc_sb)

            # ---- main schedule ----
            # pass order spreads the demoted-projection filler demand: kq2
            # is only needed by pass 4 and q01-qb1 by pass 3, so the early
            # PE idle slots aren't oversubscribed.
            with tc.high_priority():
                attn_head_pass(0, 0)
                attn_head_pass(0, 1)
                attn_head_pass(1, 0)
                attn_head_pass(0, 2)
            with tc.high_priority(**demote):
                for sub in range(8):        # qb0 blocks: filler for qb1
                    out_block_unified(sub)
            with tc.high_priority():
                attn_head_pass(1, 1)
            with tc.high_priority(**demote):
                for sub in range(8, 16):
                    out_block_ab(sub)
            with tc.high_priority():
                attn_head_pass(1, 2)
            with tc.high_priority(**demote):
                for sub in range(8, 16):
                    out_block_c(sub)

    nc.compile()
    return nc


def _prep_core_inputs(x, Wq, bq, Wk, bk, Wv, bv, Wo, bo, core):
    b, g = divmod(core, 4)
    cs = slice(g * DC, (g + 1) * DC)
    xTb = np.ascontiguousarray(x[b].T).astype(BF16)
    Wq_c, Wk_c, Wv_c = Wq[:, cs], Wk[:, cs], Wv[:, cs]
    # wqkv columns: [k01 | q01 | v | k2 q2]
    wqkv = np.concatenate(
        [Wk_c[:, :128], Wq_c[:, :128], Wv_c, Wk_c[:, 128:], Wq_c[:, 128:]],
        axis=1).astype(BF16)
    wo_c = Wo[cs, :].reshape(HPC, HD, D).transpose(1, 0, 2)  # (HD, HPC, D)
    bq_c, bk_c = bq[cs], bk[cs]
    bqk0 = np.stack([bq_c[:128], bk_c[:128]], axis=1).astype(np.float32)
    bkq2 = np.concatenate([bk_c[128:], bq_c[128:]]).reshape(128, 1)
    bo_t = (np.broadcast_to(bo, (128, D)) if g == 0
            else np.zeros((128, D), np.float32))
    return {
        "xT": xTb,
        "wqkv": np.ascontiguousarray(wqkv),
        "wo": np.ascontiguousarray(wo_c).astype(BF16),
        "bqk0": np.ascontiguousarray(bqk0),
        "bkq2": np.ascontiguousarray(bkq2).astype(np.float32),
        "bv": np.ascontiguousarray(bv[cs]).reshape(1, DC).astype(BF16),
        "bo_t": np.ascontiguousarray(bo_t).astype(np.float32),
    }


def kernel(x, Wq, bq, Wk, bk, Wv, bv, Wo, bo, _trace=False):
    from concourse.bass_utils import run_bass_kernel_spmd

    x = np.asarray(x, np.float32)
    Wq, bq = np.asarray(Wq, np.float32), np.asarray(bq, np.float32)
    Wk, bk = np.asarray(Wk, np.float32), np.asarray(bk, np.float32)
    Wv, bv = np.asarray(Wv, np.float32), np.asarray(bv, np.float32)
    Wo, bo = np.asarray(Wo, np.float32), np.asarray(bo, np.float32)

    if "nc" not in _cache:
        _cache["nc"] = _build_nc()
    nc = _cache["nc"]

    in_maps = [_prep_core_inputs(x, Wq, bq, Wk, bk, Wv, bv, Wo, bo, c)
               for c in range(8)]
    res = run_bass_kernel_spmd(nc, in_maps, core_ids=list(range(8)),
                               trace=_trace)
    _cache["last_result"] = res
    parts = [np.asarray(r["out"], dtype=np.float32) for r in res.results]
    full = np.zeros((B, S, D), np.float32)
    for b in range(B):
        full[b] = parts[4 * b] + parts[4 * b + 1] + parts[4 * b + 2] + parts[4 * b + 3]
    return full
